# revision 3
# baseline (speedup 1.0000x reference)
"""Trainium2 Bass kernel for nn_Attention_35588099015470.

Full transformer attention block: LoRA linears (folded host-side) + RoPE +
causal SDPA + output projection, B=2 T=2048 C=2048 H=16 D=128, fp32 in/out.

Sharding: tensor-parallel over heads — 8 cores x 2 heads. All matmul operands
are bf16 (fp32 PSUM accumulation): same PE rate as fp32r on this hardware but
half the DMA/SBUF footprint, which lets q/k/v live entirely in SBUF between
the projection phase and attention (no DRAM spill round-trip).

Phase A computes q/k/v for the core's 2 heads in transposed [feature, token]
layout (RoPE fused on the DVE), writing straight into persistent SBUF tiles;
v is PE-transposed to natural [token, d] layout for the PV stationary.
Phase B runs causal attention per (batch, head) in [key, query] score layout:
scores + lstep/rmask causal-mask matmuls + exp on the Act engine + ones-matmul
column sums + PV, all software-pipelined on the PE. Normalization is two DVE
ops per query tile (reciprocal of the full colsum PSUM tile - every partition
already holds the sum - then multiply), feeding an AllToAll per (batch, head)
that reshards head-parallel -> token-parallel. Pair order (0,0),(1,0),(0,1),
(1,1) lets the hl=0 half of the output projection run between attention pairs
(partials kept in bf16 SBUF), so only the hl=1 half remains after the last
AllToAll - that shortens the collective-exposed tail.

Biases are guaranteed zero by the problem's setup_inputs and the mask is the
causal tril; if either assumption is violated at runtime we fall back to a
host reference implementation so the kernel stays correct on any input.
"""
import sys

sys.path.insert(0, "/opt/trn_rl_repo")

import numpy as np
import ml_dtypes
from contextlib import ExitStack

import concourse.tile as tile
from concourse import bacc, mybir
from concourse.bass_utils import run_bass_kernel_spmd

dt = mybir.dt
MMDT = dt.bfloat16

B, T, C, H, R = 2, 2048, 2048, 16, 8
D = C // H            # 128
NCORES = 8
HPC = H // NCORES     # heads per core = 2
P = 128
TT = (B * T) // 512   # 8 token tiles of 512
KC = C // P           # 16 contraction chunks
QT = T // 512         # 4 query tiles per (b, h)
SCALE = 1.0 / float(np.sqrt(D))
BT = B * T

_PROGRAM = None


def _build_program():
    nc = bacc.Bacc("TRN2", target_bir_lowering=False, debug=False,
                   num_devices=NCORES)

    xT_d = nc.dram_tensor("xT", [C, BT], MMDT, kind="ExternalInput")
    wqT_d = nc.dram_tensor("wqT", [C, HPC * D], MMDT, kind="ExternalInput")
    wkT_d = nc.dram_tensor("wkT", [C, HPC * D], MMDT, kind="ExternalInput")
    wvT_d = nc.dram_tensor("wvT", [C, HPC * D], MMDT, kind="ExternalInput")
    # [co, hl, p, r, m]: phase-C weights, hl-major so each half streams whole
    pwB_d = nc.dram_tensor("pwB", [KC, HPC, P, NCORES, P], MMDT,
                           kind="ExternalInput")
    cosA_d = nc.dram_tensor("cosA", [P, BT], dt.float32, kind="ExternalInput")
    sinA_d = nc.dram_tensor("sinA", [P, BT], dt.float32, kind="ExternalInput")
    lstep_d = nc.dram_tensor("lstep", [P, P], MMDT, kind="ExternalInput")
    rmask_d = nc.dram_tensor("rmask", [4, P, 512], MMDT, kind="ExternalInput")
    ident_d = nc.dram_tensor("ident", [P, P], MMDT, kind="ExternalInput")

    outT_d = nc.dram_tensor("outT", [C, 512], dt.float32, kind="ExternalOutput")

    with tile.TileContext(nc) as tc, ExitStack() as ctx:
        dram = ctx.enter_context(tc.tile_pool(name="dram", bufs=1, space="DRAM"))
        # A2A staging: one collective per (batch, head-local)
        chs = [[dram.tile([NCORES, D, 256], MMDT, name=f"ch_{b}_{hl}")
                for hl in range(HPC)] for b in range(B)]
        yos = [[dram.tile([NCORES * D, 256], MMDT, name=f"yo_{b}_{hl}")
                for hl in range(HPC)] for b in range(B)]

        # persistent SBUF (live across phases)
        cst = ctx.enter_context(tc.tile_pool(name="cst", bufs=1))
        qT_all = cst.tile([P, HPC, BT], MMDT, name="qT_all")
        kT_all = cst.tile([P, HPC, BT], MMDT, name="kT_all")
        v_all = cst.tile([P, TT * 4, HPC * D], MMDT, name="v_all")
        cpart = cst.tile([P, KC, 512], MMDT, name="cpart")
        yAB = cst.tile([P, HPC, NCORES, 512], MMDT, name="yAB")

        ones_f = cst.tile([P, P], dt.float32, name="ones_f")
        nc.any.memset(ones_f[:], 1.0)
        ones_r = cst.tile([P, P], MMDT, name="ones_r")
        nc.vector.tensor_copy(ones_r[:], ones_f[:])
        ident = cst.tile([P, P], MMDT, name="ident")
        lstep = cst.tile([P, P], MMDT, name="lstep")
        rmask = cst.tile([P, 4, 512], MMDT, name="rmask")

        # ---------------- Phase A: q/k/v projections + RoPE -----------------
        with tc.tile_pool(name="pa_w", bufs=1) as wp, \
             tc.tile_pool(name="pa_x", bufs=2) as xp, \
             tc.tile_pool(name="pa_cs", bufs=3) as csp, \
             tc.tile_pool(name="pa_tmp", bufs=3) as tp, \
             tc.tile_pool(name="pa_vt", bufs=3) as vtp, \
             tc.tile_pool(name="pa_ps", bufs=1, space="PSUM") as pp:

            xT_view = xT_d.ap().rearrange("(a p) t -> p a t", p=P)
            wq_sb = wp.tile([P, KC, HPC * D], MMDT, name="wq_sb")
            wk_sb = wp.tile([P, KC, HPC * D], MMDT, name="wk_sb")
            wv_sb = wp.tile([P, KC, HPC * D], MMDT, name="wv_sb")
            # first x group + first wq group first, so the PE starts ASAP
            xt0 = xp.tile([P, KC, 512], MMDT, name="xt_0", tag="xt")
            wq_view = wqT_d.ap().rearrange("(a p) m -> p a m", p=P)
            nc.sync.dma_start(xt0[:, 0:4, :], xT_view[:, 0:4, 0:512])
            nc.sync.dma_start(wq_sb[:, 0:4, :], wq_view[:, 0:4, :])
            for g in range(1, 4):
                nc.sync.dma_start(xt0[:, g * 4:(g + 1) * 4, :],
                                  xT_view[:, g * 4:(g + 1) * 4, 0:512])
                nc.sync.dma_start(wq_sb[:, g * 4:(g + 1) * 4, :],
                                  wq_view[:, g * 4:(g + 1) * 4, :])
            for w_sb, wd in ((wk_sb, wkT_d), (wv_sb, wvT_d)):
                wv_view = wd.ap().rearrange("(a p) m -> p a m", p=P)
                for g in range(4):
                    nc.sync.dma_start(w_sb[:, g * 4:(g + 1) * 4, :],
                                      wv_view[:, g * 4:(g + 1) * 4, :])
            nc.sync.dma_start(ident[:], ident_d.ap())
            nc.sync.dma_start(lstep[:], lstep_d.ap())
            for o in range(4):
                nc.sync.dma_start(rmask[:, o, :], rmask_d.ap()[o])

            for tt in range(TT):
                tsl = slice(tt * 512, (tt + 1) * 512)
                if tt == 0:
                    xt = xt0
                else:
                    xt = xp.tile([P, KC, 512], MMDT, name=f"xt_{tt}", tag="xt")
                    for g in range(4):
                        nc.sync.dma_start(xt[:, g * 4:(g + 1) * 4, :],
                                          xT_view[:, g * 4:(g + 1) * 4, tsl])
                cs_c = csp.tile([P, 512], dt.float32, tag="csc", name=f"csc_{tt}")
                nc.sync.dma_start(cs_c[:], cosA_d.ap()[:, tsl])
                cs_s = csp.tile([P, 512], dt.float32, tag="css", name=f"css_{tt}")
                nc.sync.dma_start(cs_s[:], sinA_d.ap()[:, tsl])

                for w_sb, dst in ((wq_sb, qT_all), (wk_sb, kT_all)):
                    for mt in range(HPC):
                        ps = pp.tile([P, 512], dt.float32, tag="qk", bufs=6,
                                     name=f"psA_{tt}_{mt}")
                        for kc in range(KC):
                            nc.tensor.matmul(
                                ps[:], w_sb[:, kc, mt * P:(mt + 1) * P],
                                xt[:, kc, :],
                                start=(kc == 0), stop=(kc == KC - 1))
                        # rope: y = raw*cosA + halfswap(raw)*sinA
                        t1 = tp.tile([P, 512], dt.float32, tag="t1",
                                     name=f"t1_{tt}_{mt}")
                        nc.vector.tensor_mul(t1[:], ps[:], cs_c[:])
                        t2 = tp.tile([P, 512], dt.float32, tag="t2",
                                     name=f"t2_{tt}_{mt}")
                        nc.vector.tensor_mul(t2[0:64, :], ps[64:128, :],
                                             cs_s[0:64, :])
                        nc.vector.tensor_mul(t2[64:128, :], ps[0:64, :],
                                             cs_s[64:128, :])
                        nc.vector.tensor_add(dst[:, mt, tsl], t1[:], t2[:])

                # v computed transposed (N=512 matmuls), then PE-transposed
                # back to natural [token, d] layout for the PV stationary
                for mt in range(HPC):
                    ps = pp.tile([P, 512], dt.float32, tag="qk", bufs=6,
                                 name=f"psVT_{tt}_{mt}")
                    for kc in range(KC):
                        nc.tensor.matmul(
                            ps[:], wv_sb[:, kc, mt * P:(mt + 1) * P],
                            xt[:, kc, :],
                            start=(kc == 0), stop=(kc == KC - 1))
                    vT_sb = vtp.tile([P, 512], MMDT, tag="vts",
                                     name=f"vts_{tt}_{mt}")
                    nc.scalar.copy(vT_sb[:], ps[:])
                    for js in range(4):
                        pst = pp.tile([P, P], MMDT, tag="tp", bufs=2,
                                      name=f"pst_{tt}_{mt}_{js}")
                        nc.tensor.transpose(pst[:], vT_sb[:, js * P:(js + 1) * P],
                                            ident[:])
                        dst_v = v_all[:, tt * 4 + js, mt * P:(mt + 1) * P]
                        if js % 2 == 0:
                            nc.scalar.copy(dst_v, pst[:])
                        else:
                            nc.vector.tensor_copy(dst_v, pst[:])

        # ---------------- Phase B + C: attention, A2A, out-projection ------
        with tc.tile_pool(name="pb_p", bufs=6) as ppool, \
             tc.tile_pool(name="pb_nm", bufs=2) as nmp, \
             tc.tile_pool(name="pc_w", bufs=4) as pwp, \
             tc.tile_pool(name="pc_o", bufs=3) as ocp, \
             tc.tile_pool(name="pb_ps", bufs=1, space="PSUM") as pb:

            def attention_pair(b, hl):
                kT_h = kT_all[:, hl, b * T:(b + 1) * T]
                qT_h = qT_all[:, hl, b * T:(b + 1) * T]
                v_h = v_all[:, b * 16:(b + 1) * 16, hl * D:(hl + 1) * D]

                for qt in range(QT):
                    qTt = qT_h[:, qt * 512:(qt + 1) * 512]
                    n = 4 * (qt + 1)
                    smps = pb.tile([P, 512], dt.float32, tag="sm", bufs=2,
                                   name=f"sm_{b}_{hl}_{qt}")
                    pvps = pb.tile([P, 512], dt.float32, tag="pv", bufs=2,
                                   name=f"pv_{b}_{hl}_{qt}")

                    sc_tiles = {}

                    def emit_sc(jc, _qt=qt, _q=qTt, _n=n, _sc=sc_tiles,
                                _b=b, _hl=hl):
                        ps = pb.tile([P, 512], dt.float32, tag="sc", bufs=3,
                                     name=f"sc_{_b}_{_hl}_{_qt}_{jc}")
                        diag = jc >= _n - 4
                        nc.tensor.matmul(ps[:], kT_h[:, jc * P:(jc + 1) * P],
                                         _q[:], start=True, stop=not diag)
                        if diag:
                            o = jc - (_n - 4)
                            nc.tensor.matmul(ps[:], lstep[:], rmask[:, o, :],
                                             start=False, stop=True)
                        _sc[jc] = ps

                    emit_sc(0)
                    if n > 1:
                        emit_sc(1)
                    for jc in range(n):
                        scps = sc_tiles.pop(jc)
                        pT = ppool.tile([P, 512], MMDT, tag="pT",
                                        name=f"pT_{b}_{hl}_{qt}_{jc}")
                        nc.scalar.activation(pT[:], scps[:],
                                             mybir.ActivationFunctionType.Exp,
                                             scale=SCALE)
                        if jc + 2 < n:
                            emit_sc(jc + 2)
                        nc.tensor.matmul(smps[:], ones_r[:], pT[:],
                                         start=(jc == 0), stop=(jc == n - 1))
                        nc.tensor.matmul(pvps[:], v_h[:, jc, :], pT[:],
                                         start=(jc == 0), stop=(jc == n - 1))

                    # normalize: all 128 partitions of smps hold the colsum,
                    # so reciprocal of the full tile is a pre-broadcast scale
                    rec = nmp.tile([P, 512], dt.float32, tag="rec",
                                   name=f"rec_{b}_{hl}_{qt}")
                    nc.vector.reciprocal(rec[:], smps[:])
                    yt = nmp.tile([P, 512], MMDT, tag="yt",
                                  name=f"yt_{b}_{hl}_{qt}")
                    nc.vector.tensor_mul(yt[:], pvps[:], rec[:])
                    nc.sync.dma_start(chs[b][hl][2 * qt][:, :], yt[:, 0:256])
                    nc.sync.dma_start(chs[b][hl][2 * qt + 1][:, :],
                                      yt[:, 256:512])

                nc.gpsimd.collective_compute(
                    "AllToAll", mybir.AluOpType.bypass,
                    replica_groups=[list(range(NCORES))],
                    ins=[chs[b][hl].opt()], outs=[yos[b][hl].opt()],
                )
                # gather this pair's slices into yAB as soon as the A2A lands
                yv = yos[b][hl][:].rearrange("(r p) t -> p r t", p=P)
                for r in range(NCORES):
                    nc.sync.dma_start(yAB[:, hl, r, b * 256:(b + 1) * 256],
                                      yv[:, r, :])

            def cpass(hlh, first):
                """Half of the output projection (head-local hlh of every
                rank). first=True stores partials; else adds and emits."""
                for co in range(KC):
                    pw = pwp.tile([P, NCORES, P], MMDT, tag="pw",
                                  name=f"pw_{hlh}_{co}")
                    nc.sync.dma_start(pw[:], pwB_d.ap()[co, hlh])
                    pso = pb.tile([P, 512], dt.float32, tag="sc", bufs=3,
                                  name=f"pso_{hlh}_{co}")
                    for r in range(NCORES):
                        nc.tensor.matmul(pso[:], pw[:, r, :], yAB[:, hlh, r, :],
                                         start=(r == 0), stop=(r == NCORES - 1))
                    if first:
                        if co % 2 == 0:
                            nc.scalar.copy(cpart[:, co, :], pso[:])
                        else:
                            nc.vector.tensor_copy(cpart[:, co, :], pso[:])
                    else:
                        oo = ocp.tile([P, 512], dt.float32, tag="oo",
                                      name=f"oo_{co}")
                        nc.vector.tensor_add(oo[:], pso[:], cpart[:, co, :])
                        nc.sync.dma_start(outT_d.ap()[co * P:(co + 1) * P, :],
                                          oo[:])

            attention_pair(0, 0)
            attention_pair(1, 0)
            attention_pair(0, 1)
            cpass(0, first=True)       # hl=0 half, hidden behind pair (1,1)
            attention_pair(1, 1)
            cpass(1, first=False)      # hl=1 half + combine, collective tail

    nc.compile()
    return nc


def _host_reference(x, weights, cos, sin, mask, use_lora):
    """Numpy fallback for inputs outside the optimized assumptions."""
    (q_w, q_b, q_A, q_B, k_w, k_b, k_A, k_B,
     v_w, v_b, v_A, v_B, p_w, p_b, p_A, p_B) = weights

    def lin(xx, w, b, A, Bm):
        out = xx @ w.T + b
        if use_lora:
            out = out + (xx @ A) @ Bm
        return out

    def rope(t):
        x1, x2 = t[..., ::2], t[..., 1::2]
        y = np.stack((x1 * cos - x2 * sin, x1 * sin + x2 * cos), axis=-1)
        return y.reshape(t.shape)

    Bs, Tl, Cd = x.shape
    q = lin(x, q_w, q_b, q_A, q_B).reshape(Bs, Tl, H, D).transpose(0, 2, 1, 3)
    k = lin(x, k_w, k_b, k_A, k_B).reshape(Bs, Tl, H, D).transpose(0, 2, 1, 3)
    v = lin(x, v_w, v_b, v_A, v_B).reshape(Bs, Tl, H, D).transpose(0, 2, 1, 3)
    q, k = rope(q), rope(k)
    s = np.einsum('bhqd,bhkd->bhqk', q, k) / np.sqrt(D)
    s = np.where(mask, s, -np.inf)
    s = s - s.max(axis=-1, keepdims=True)
    p = np.exp(s)
    p /= p.sum(axis=-1, keepdims=True)
    o = np.einsum('bhqk,bhkd->bhqd', p, v).transpose(0, 2, 1, 3).reshape(Bs, Tl, Cd)
    return lin(o, p_w, p_b, p_A, p_B).astype(np.float32)


def kernel(**inputs):
    x = np.asarray(inputs["x"], np.float32)
    cos = np.asarray(inputs["cos"], np.float32)
    sin = np.asarray(inputs["sin"], np.float32)
    mask = np.asarray(inputs["mask"])
    use_lora = int(np.asarray(inputs["use_lora"]))
    ws = {}
    for nm in ("q", "k", "v", "p"):
        for suf in ("w", "b", "A", "B"):
            ws[f"{nm}_{suf}"] = np.asarray(inputs[f"{nm}_{suf}"], np.float32)

    causal = bool((mask == np.tril(np.ones((T, T), bool))).all())
    zero_bias = all(not ws[f"{nm}_b"].any() for nm in ("q", "k", "v", "p"))
    if not (causal and zero_bias and x.shape == (B, T, C)):
        weights = tuple(ws[f"{nm}_{suf}"] for nm in ("q", "k", "v", "p")
                        for suf in ("w", "b", "A", "B"))
        return _host_reference(x, weights, cos, sin, mask, use_lora)

    # effective (LoRA-folded) transposed weights: out = x @ W_eff.T,
    # W_eff.T = w.T + A @ B
    effT = {}
    for nm in ("q", "k", "v", "p"):
        wt = ws[f"{nm}_w"].T.copy()
        if use_lora:
            wt += ws[f"{nm}_A"] @ ws[f"{nm}_B"]
        effT[nm] = np.ascontiguousarray(wt, np.float32)

    xT = np.ascontiguousarray(x.reshape(BT, C).T)

    # sigma: within each head reorder out-features to [evens, odds] so the
    # rope pair-rotation becomes a partition half-swap
    perm = np.concatenate([np.arange(0, D, 2), np.arange(1, D, 2)])
    cosT = cos.T.astype(np.float32)          # [64, T]
    sinT = sin.T.astype(np.float32)
    cosA = np.tile(np.vstack([cosT, cosT]), (1, B))          # [128, B*T]
    sinA = np.tile(np.vstack([-sinT, sinT]), (1, B))

    # additive causal mask factorization: M_o = lstep.T @ rmask_o where
    # M_o[j, q] = -1e9 iff j + 128*o > q (adds to scores before exp -> 0)
    lstep = np.tril(np.ones((P, P), np.float32)).T          # L[m, jr] = jr >= m
    rmask = np.zeros((4, P, 512), np.float32)
    for o in range(4):
        for qr in range(512):
            m = max(0, qr + 1 - 128 * o)     # m=0 row covers fully-masked cols
            if m < P:
                rmask[o, m, qr] = -1e9

    # output projection weight, blocked [co, hl, p, r, m]: y-feature block for
    # head (r, hl) contracts against rows (2r+hl)*128+p of effT["p"]
    pwB = np.ascontiguousarray(
        effT["p"].reshape(NCORES, HPC, P, KC, P)     # [r, hl, p, co, m]
        .transpose(3, 1, 2, 0, 4))                   # [co, hl, p, r, m]

    ident = np.eye(P, dtype=np.float32)

    global _PROGRAM
    if _PROGRAM is None:
        _PROGRAM = _build_program()
    nc = _PROGRAM

    mmnp = mybir.dt.np(MMDT)

    in_maps = []
    xT_mm = xT.astype(mmnp)
    pwB_mm = pwB.astype(mmnp)
    for c in range(NCORES):
        cols = slice(c * HPC * D, (c + 1) * HPC * D)
        wqT = effT["q"][:, cols].copy()
        wkT = effT["k"][:, cols].copy()
        for hl in range(HPC):
            sl = slice(hl * D, (hl + 1) * D)
            wqT[:, sl] = wqT[:, sl][:, perm]
            wkT[:, sl] = wkT[:, sl][:, perm]
        in_maps.append({
            "xT": xT_mm,
            "wqT": np.ascontiguousarray(wqT).astype(mmnp),
            "wkT": np.ascontiguousarray(wkT).astype(mmnp),
            "wvT": np.ascontiguousarray(effT["v"][:, cols]).astype(mmnp),
            "pwB": pwB_mm,
            "cosA": cosA,
            "sinA": sinA,
            "lstep": lstep.astype(mmnp),
            "rmask": rmask.astype(mmnp),
            "ident": ident.astype(mmnp),
        })

    res = run_bass_kernel_spmd(nc, in_maps, list(range(NCORES)))

    out = np.empty((BT, C), np.float32)
    for c in range(NCORES):
        oT = res.results[c]["outT"]                    # [2048, 512]
        out[c * 256:(c + 1) * 256, :] = oT[:, 0:256].T             # b = 0
        out[T + c * 256:T + (c + 1) * 256, :] = oT[:, 256:512].T   # b = 1
    return out.reshape(B, T, C)


# revision 4
# speedup vs baseline: 1.2120x; 1.2120x over previous
"""Trainium2 Bass kernel for nn_Attention_35588099015470.

Full transformer attention block: LoRA linears (folded host-side) + RoPE +
causal SDPA + output projection, B=2 T=2048 C=2048 H=16 D=128, fp32 in/out.

Sharding: tensor-parallel over heads — 8 cores x 2 heads. All matmul operands
are bf16 (fp32 PSUM accumulation): same PE rate as fp32r on this hardware but
half the DMA/SBUF footprint, which lets q/k/v live entirely in SBUF between
the projection phase and attention (no DRAM spill round-trip).

Phase A computes q/k/v for the core's 2 heads in transposed [feature, token]
layout (RoPE fused on the DVE), writing straight into persistent SBUF tiles;
v is PE-transposed to natural [token, d] layout for the PV stationary.
Phase B runs causal attention per (batch, head) in [key, query] score layout:
scores + lstep/rmask causal-mask matmuls + exp on the Act engine + ones-matmul
column sums + PV, all software-pipelined on the PE. Normalization is two DVE
ops per query tile (reciprocal of the full colsum PSUM tile - every partition
already holds the sum - then multiply), feeding an AllToAll per (batch, head)
that reshards head-parallel -> token-parallel. Pair order (0,0),(1,0),(0,1),
(1,1) lets the hl=0 half of the output projection run between attention pairs
(partials kept in bf16 SBUF), so only the hl=1 half remains after the last
AllToAll - that shortens the collective-exposed tail.

Biases are guaranteed zero by the problem's setup_inputs and the mask is the
causal tril; if either assumption is violated at runtime we fall back to a
host reference implementation so the kernel stays correct on any input.
"""
import sys

sys.path.insert(0, "/opt/trn_rl_repo")

import numpy as np
import ml_dtypes
from contextlib import ExitStack

import concourse.tile as tile
from concourse import bacc, mybir
from concourse.bass_utils import run_bass_kernel_spmd

dt = mybir.dt
MMDT = dt.bfloat16

B, T, C, H, R = 2, 2048, 2048, 16, 8
D = C // H            # 128
NCORES = 8
HPC = H // NCORES     # heads per core = 2
P = 128
TT = (B * T) // 512   # 8 token tiles of 512
KC = C // P           # 16 contraction chunks
QT = T // 512         # 4 query tiles per (b, h)
SCALE = 1.0 / float(np.sqrt(D))
BT = B * T

_PROGRAM = None


def _build_program():
    nc = bacc.Bacc("TRN2", target_bir_lowering=False, debug=False,
                   num_devices=NCORES)

    xT_d = nc.dram_tensor("xT", [C, BT], MMDT, kind="ExternalInput")
    wqT_d = nc.dram_tensor("wqT", [C, HPC * D], MMDT, kind="ExternalInput")
    wkT_d = nc.dram_tensor("wkT", [C, HPC * D], MMDT, kind="ExternalInput")
    wvT_d = nc.dram_tensor("wvT", [C, HPC * D], MMDT, kind="ExternalInput")
    # [co, hl, p, r, m]: phase-C weights, hl-major so each half streams whole
    pwB_d = nc.dram_tensor("pwB", [KC, HPC, P, NCORES, P], MMDT,
                           kind="ExternalInput")
    cosA_d = nc.dram_tensor("cosA", [P, BT], dt.float32, kind="ExternalInput")
    sinA_d = nc.dram_tensor("sinA", [P, BT], dt.float32, kind="ExternalInput")
    lstep_d = nc.dram_tensor("lstep", [P, P], MMDT, kind="ExternalInput")
    rmask_d = nc.dram_tensor("rmask", [4, P, 512], MMDT, kind="ExternalInput")
    ident_d = nc.dram_tensor("ident", [P, P], MMDT, kind="ExternalInput")

    outT_d = nc.dram_tensor("outT", [C, 512], dt.float32, kind="ExternalOutput")

    with tile.TileContext(nc) as tc, ExitStack() as ctx:
        dram = ctx.enter_context(tc.tile_pool(name="dram", bufs=1, space="DRAM"))
        # A2A staging: one collective per (batch, head-local)
        chs = [[dram.tile([NCORES, D, 256], MMDT, name=f"ch_{b}_{hl}")
                for hl in range(HPC)] for b in range(B)]
        yos = [[dram.tile([NCORES * D, 256], MMDT, name=f"yo_{b}_{hl}")
                for hl in range(HPC)] for b in range(B)]

        # persistent SBUF (live across phases)
        cst = ctx.enter_context(tc.tile_pool(name="cst", bufs=1))
        qT_all = cst.tile([P, HPC, BT], MMDT, name="qT_all")
        kT_all = cst.tile([P, HPC, BT], MMDT, name="kT_all")
        v_all = cst.tile([P, TT * 4, HPC * D], MMDT, name="v_all")
        cpart = cst.tile([P, KC, 512], MMDT, name="cpart")
        yAB = cst.tile([P, HPC, NCORES, 512], MMDT, name="yAB")

        ones_f = cst.tile([P, P], dt.float32, name="ones_f")
        nc.any.memset(ones_f[:], 1.0)
        ones_r = cst.tile([P, P], MMDT, name="ones_r")
        nc.vector.tensor_copy(ones_r[:], ones_f[:])
        ident = cst.tile([P, P], MMDT, name="ident")
        lstep = cst.tile([P, P], MMDT, name="lstep")
        rmask = cst.tile([P, 4, 512], MMDT, name="rmask")

        # ---------------- Phase A: q/k/v projections + RoPE -----------------
        with tc.tile_pool(name="pa_w", bufs=1) as wp, \
             tc.tile_pool(name="pa_x", bufs=2) as xp, \
             tc.tile_pool(name="pa_cs", bufs=3) as csp, \
             tc.tile_pool(name="pa_tmp", bufs=3) as tp, \
             tc.tile_pool(name="pa_vt", bufs=3) as vtp, \
             tc.tile_pool(name="pa_ps", bufs=1, space="PSUM") as pp:

            xT_view = xT_d.ap().rearrange("(a p) t -> p a t", p=P)
            wq_sb = wp.tile([P, KC, HPC * D], MMDT, name="wq_sb")
            wk_sb = wp.tile([P, KC, HPC * D], MMDT, name="wk_sb")
            wv_sb = wp.tile([P, KC, HPC * D], MMDT, name="wv_sb")
            # first x group + first wq group first, so the PE starts ASAP
            xt0 = xp.tile([P, KC, 512], MMDT, name="xt_0", tag="xt")
            wq_view = wqT_d.ap().rearrange("(a p) m -> p a m", p=P)
            nc.sync.dma_start(xt0[:, 0:4, :], xT_view[:, 0:4, 0:512])
            nc.sync.dma_start(wq_sb[:, 0:4, :], wq_view[:, 0:4, :])
            for g in range(1, 4):
                nc.sync.dma_start(xt0[:, g * 4:(g + 1) * 4, :],
                                  xT_view[:, g * 4:(g + 1) * 4, 0:512])
                nc.sync.dma_start(wq_sb[:, g * 4:(g + 1) * 4, :],
                                  wq_view[:, g * 4:(g + 1) * 4, :])
            for w_sb, wd in ((wk_sb, wkT_d), (wv_sb, wvT_d)):
                wv_view = wd.ap().rearrange("(a p) m -> p a m", p=P)
                for g in range(4):
                    nc.sync.dma_start(w_sb[:, g * 4:(g + 1) * 4, :],
                                      wv_view[:, g * 4:(g + 1) * 4, :])
            nc.sync.dma_start(ident[:], ident_d.ap())
            nc.sync.dma_start(lstep[:], lstep_d.ap())
            for o in range(4):
                nc.sync.dma_start(rmask[:, o, :], rmask_d.ap()[o])

            for tt in range(TT):
                tsl = slice(tt * 512, (tt + 1) * 512)
                if tt == 0:
                    xt = xt0
                else:
                    xt = xp.tile([P, KC, 512], MMDT, name=f"xt_{tt}", tag="xt")
                    for g in range(4):
                        nc.sync.dma_start(xt[:, g * 4:(g + 1) * 4, :],
                                          xT_view[:, g * 4:(g + 1) * 4, tsl])
                cs_c = csp.tile([P, 512], dt.float32, tag="csc", name=f"csc_{tt}")
                nc.sync.dma_start(cs_c[:], cosA_d.ap()[:, tsl])
                cs_s = csp.tile([P, 512], dt.float32, tag="css", name=f"css_{tt}")
                nc.sync.dma_start(cs_s[:], sinA_d.ap()[:, tsl])

                for w_sb, dst in ((wq_sb, qT_all), (wk_sb, kT_all)):
                    for mt in range(HPC):
                        ps = pp.tile([P, 512], dt.float32, tag="qk", bufs=6,
                                     name=f"psA_{tt}_{mt}")
                        for kc in range(KC):
                            nc.tensor.matmul(
                                ps[:], w_sb[:, kc, mt * P:(mt + 1) * P],
                                xt[:, kc, :],
                                start=(kc == 0), stop=(kc == KC - 1))
                        # rope: y = raw*cosA + halfswap(raw)*sinA
                        t1 = tp.tile([P, 512], dt.float32, tag="t1",
                                     name=f"t1_{tt}_{mt}")
                        nc.vector.tensor_mul(t1[:], ps[:], cs_c[:])
                        t2 = tp.tile([P, 512], dt.float32, tag="t2",
                                     name=f"t2_{tt}_{mt}")
                        nc.vector.tensor_mul(t2[0:64, :], ps[64:128, :],
                                             cs_s[0:64, :])
                        nc.vector.tensor_mul(t2[64:128, :], ps[0:64, :],
                                             cs_s[64:128, :])
                        nc.vector.tensor_add(dst[:, mt, tsl], t1[:], t2[:])

                # v computed transposed (N=512 matmuls), then PE-transposed
                # back to natural [token, d] layout for the PV stationary
                for mt in range(HPC):
                    ps = pp.tile([P, 512], dt.float32, tag="qk", bufs=6,
                                 name=f"psVT_{tt}_{mt}")
                    for kc in range(KC):
                        nc.tensor.matmul(
                            ps[:], wv_sb[:, kc, mt * P:(mt + 1) * P],
                            xt[:, kc, :],
                            start=(kc == 0), stop=(kc == KC - 1))
                    vT_sb = vtp.tile([P, 512], MMDT, tag="vts",
                                     name=f"vts_{tt}_{mt}")
                    nc.scalar.copy(vT_sb[:], ps[:])
                    for js in range(4):
                        pst = pp.tile([P, P], MMDT, tag="tp", bufs=2,
                                      name=f"pst_{tt}_{mt}_{js}")
                        nc.tensor.transpose(pst[:], vT_sb[:, js * P:(js + 1) * P],
                                            ident[:])
                        dst_v = v_all[:, tt * 4 + js, mt * P:(mt + 1) * P]
                        if js % 2 == 0:
                            nc.scalar.copy(dst_v, pst[:])
                        else:
                            nc.vector.tensor_copy(dst_v, pst[:])

        # ---------------- Phase B + C: attention, A2A, out-projection ------
        with tc.tile_pool(name="pb_p", bufs=6) as ppool, \
             tc.tile_pool(name="pb_nm", bufs=2) as nmp, \
             tc.tile_pool(name="pc_w", bufs=4) as pwp, \
             tc.tile_pool(name="pc_o", bufs=3) as ocp, \
             tc.tile_pool(name="pb_ps", bufs=1, space="PSUM") as pb:

            def attention_pair(b, hl):
                kT_h = kT_all[:, hl, b * T:(b + 1) * T]
                qT_h = qT_all[:, hl, b * T:(b + 1) * T]
                v_h = v_all[:, b * 16:(b + 1) * 16, hl * D:(hl + 1) * D]

                for qt in range(QT):
                    qTt = qT_h[:, qt * 512:(qt + 1) * 512]
                    n = 4 * (qt + 1)
                    smps = pb.tile([P, 512], dt.float32, tag="sm", bufs=2,
                                   name=f"sm_{b}_{hl}_{qt}")
                    pvps = pb.tile([P, 512], dt.float32, tag="pv", bufs=2,
                                   name=f"pv_{b}_{hl}_{qt}")

                    sc_tiles = {}

                    def emit_sc(jc, _qt=qt, _q=qTt, _n=n, _sc=sc_tiles,
                                _b=b, _hl=hl):
                        ps = pb.tile([P, 512], dt.float32, tag="sc", bufs=3,
                                     name=f"sc_{_b}_{_hl}_{_qt}_{jc}")
                        diag = jc >= _n - 4
                        nc.tensor.matmul(ps[:], kT_h[:, jc * P:(jc + 1) * P],
                                         _q[:], start=True, stop=not diag)
                        if diag:
                            o = jc - (_n - 4)
                            nc.tensor.matmul(ps[:], lstep[:], rmask[:, o, :],
                                             start=False, stop=True)
                        _sc[jc] = ps

                    emit_sc(0)
                    if n > 1:
                        emit_sc(1)
                    for jc in range(n):
                        scps = sc_tiles.pop(jc)
                        pT = ppool.tile([P, 512], MMDT, tag="pT",
                                        name=f"pT_{b}_{hl}_{qt}_{jc}")
                        nc.scalar.activation(pT[:], scps[:],
                                             mybir.ActivationFunctionType.Exp,
                                             scale=SCALE)
                        if jc + 2 < n:
                            emit_sc(jc + 2)
                        nc.tensor.matmul(smps[:], ones_r[:], pT[:],
                                         start=(jc == 0), stop=(jc == n - 1))
                        nc.tensor.matmul(pvps[:], v_h[:, jc, :], pT[:],
                                         start=(jc == 0), stop=(jc == n - 1))

                    # normalize: all 128 partitions of smps hold the colsum,
                    # so reciprocal of the full tile is a pre-broadcast scale
                    rec = nmp.tile([P, 512], dt.float32, tag="rec",
                                   name=f"rec_{b}_{hl}_{qt}")
                    nc.vector.reciprocal(rec[:], smps[:])
                    yt = nmp.tile([P, 512], MMDT, tag="yt",
                                  name=f"yt_{b}_{hl}_{qt}")
                    nc.vector.tensor_mul(yt[:], pvps[:], rec[:])
                    nc.sync.dma_start(chs[b][hl][2 * qt][:, :], yt[:, 0:256])
                    nc.sync.dma_start(chs[b][hl][2 * qt + 1][:, :],
                                      yt[:, 256:512])

                nc.gpsimd.collective_compute(
                    "AllToAll", mybir.AluOpType.bypass,
                    replica_groups=[list(range(NCORES))],
                    ins=[chs[b][hl].opt()], outs=[yos[b][hl].opt()],
                )

            def emit_gathers(hlh):
                # A2A-gated: emit only where nothing latency-critical queues
                # behind them on the same DMA queue
                for b in range(B):
                    yv = yos[b][hlh][:].rearrange("(r p) t -> p r t", p=P)
                    for r in range(NCORES):
                        nc.sync.dma_start(
                            yAB[:, hlh, r, b * 256:(b + 1) * 256], yv[:, r, :])

            pw_sb = {}

            def load_pw(hlh):
                # ungated streaming loads; fully resident (no ring waits)
                for co in range(KC):
                    pw = pwp.tile([P, NCORES, P], MMDT, tag=f"pw{hlh}",
                                  bufs=KC, name=f"pw_{hlh}_{co}")
                    nc.sync.dma_start(pw[:], pwB_d.ap()[co, hlh])
                    pw_sb[(hlh, co)] = pw

            def cpass(hlh, first):
                """Half of the output projection (head-local hlh of every
                rank). first=True stores partials; else adds and emits."""
                for co in range(KC):
                    pw = pw_sb[(hlh, co)]
                    pso = pb.tile([P, 512], dt.float32, tag="sc", bufs=3,
                                  name=f"pso_{hlh}_{co}")
                    for r in range(NCORES):
                        nc.tensor.matmul(pso[:], pw[:, r, :], yAB[:, hlh, r, :],
                                         start=(r == 0), stop=(r == NCORES - 1))
                    if first:
                        if co % 2 == 0:
                            nc.scalar.copy(cpart[:, co, :], pso[:])
                        else:
                            nc.vector.tensor_copy(cpart[:, co, :], pso[:])
                    else:
                        oo = ocp.tile([P, 512], dt.float32, tag="oo",
                                      name=f"oo_{co}")
                        nc.vector.tensor_add(oo[:], pso[:], cpart[:, co, :])
                        nc.sync.dma_start(outT_d.ap()[co * P:(co + 1) * P, :],
                                          oo[:])

            attention_pair(0, 0)
            attention_pair(1, 0)
            attention_pair(0, 1)
            load_pw(0)
            load_pw(1)
            emit_gathers(0)
            cpass(0, first=True)       # hl=0 half, hidden behind pair (1,1)
            attention_pair(1, 1)
            emit_gathers(1)
            cpass(1, first=False)      # hl=1 half + combine, collective tail

    nc.compile()
    return nc


def _host_reference(x, weights, cos, sin, mask, use_lora):
    """Numpy fallback for inputs outside the optimized assumptions."""
    (q_w, q_b, q_A, q_B, k_w, k_b, k_A, k_B,
     v_w, v_b, v_A, v_B, p_w, p_b, p_A, p_B) = weights

    def lin(xx, w, b, A, Bm):
        out = xx @ w.T + b
        if use_lora:
            out = out + (xx @ A) @ Bm
        return out

    def rope(t):
        x1, x2 = t[..., ::2], t[..., 1::2]
        y = np.stack((x1 * cos - x2 * sin, x1 * sin + x2 * cos), axis=-1)
        return y.reshape(t.shape)

    Bs, Tl, Cd = x.shape
    q = lin(x, q_w, q_b, q_A, q_B).reshape(Bs, Tl, H, D).transpose(0, 2, 1, 3)
    k = lin(x, k_w, k_b, k_A, k_B).reshape(Bs, Tl, H, D).transpose(0, 2, 1, 3)
    v = lin(x, v_w, v_b, v_A, v_B).reshape(Bs, Tl, H, D).transpose(0, 2, 1, 3)
    q, k = rope(q), rope(k)
    s = np.einsum('bhqd,bhkd->bhqk', q, k) / np.sqrt(D)
    s = np.where(mask, s, -np.inf)
    s = s - s.max(axis=-1, keepdims=True)
    p = np.exp(s)
    p /= p.sum(axis=-1, keepdims=True)
    o = np.einsum('bhqk,bhkd->bhqd', p, v).transpose(0, 2, 1, 3).reshape(Bs, Tl, Cd)
    return lin(o, p_w, p_b, p_A, p_B).astype(np.float32)


def kernel(**inputs):
    x = np.asarray(inputs["x"], np.float32)
    cos = np.asarray(inputs["cos"], np.float32)
    sin = np.asarray(inputs["sin"], np.float32)
    mask = np.asarray(inputs["mask"])
    use_lora = int(np.asarray(inputs["use_lora"]))
    ws = {}
    for nm in ("q", "k", "v", "p"):
        for suf in ("w", "b", "A", "B"):
            ws[f"{nm}_{suf}"] = np.asarray(inputs[f"{nm}_{suf}"], np.float32)

    causal = bool((mask == np.tril(np.ones((T, T), bool))).all())
    zero_bias = all(not ws[f"{nm}_b"].any() for nm in ("q", "k", "v", "p"))
    if not (causal and zero_bias and x.shape == (B, T, C)):
        weights = tuple(ws[f"{nm}_{suf}"] for nm in ("q", "k", "v", "p")
                        for suf in ("w", "b", "A", "B"))
        return _host_reference(x, weights, cos, sin, mask, use_lora)

    # effective (LoRA-folded) transposed weights: out = x @ W_eff.T,
    # W_eff.T = w.T + A @ B
    effT = {}
    for nm in ("q", "k", "v", "p"):
        wt = ws[f"{nm}_w"].T.copy()
        if use_lora:
            wt += ws[f"{nm}_A"] @ ws[f"{nm}_B"]
        effT[nm] = np.ascontiguousarray(wt, np.float32)

    xT = np.ascontiguousarray(x.reshape(BT, C).T)

    # sigma: within each head reorder out-features to [evens, odds] so the
    # rope pair-rotation becomes a partition half-swap
    perm = np.concatenate([np.arange(0, D, 2), np.arange(1, D, 2)])
    cosT = cos.T.astype(np.float32)          # [64, T]
    sinT = sin.T.astype(np.float32)
    cosA = np.tile(np.vstack([cosT, cosT]), (1, B))          # [128, B*T]
    sinA = np.tile(np.vstack([-sinT, sinT]), (1, B))

    # additive causal mask factorization: M_o = lstep.T @ rmask_o where
    # M_o[j, q] = -1e9 iff j + 128*o > q (adds to scores before exp -> 0)
    lstep = np.tril(np.ones((P, P), np.float32)).T          # L[m, jr] = jr >= m
    rmask = np.zeros((4, P, 512), np.float32)
    for o in range(4):
        for qr in range(512):
            m = max(0, qr + 1 - 128 * o)     # m=0 row covers fully-masked cols
            if m < P:
                rmask[o, m, qr] = -1e9

    # output projection weight, blocked [co, hl, p, r, m]: y-feature block for
    # head (r, hl) contracts against rows (2r+hl)*128+p of effT["p"]
    pwB = np.ascontiguousarray(
        effT["p"].reshape(NCORES, HPC, P, KC, P)     # [r, hl, p, co, m]
        .transpose(3, 1, 2, 0, 4))                   # [co, hl, p, r, m]

    ident = np.eye(P, dtype=np.float32)

    global _PROGRAM
    if _PROGRAM is None:
        _PROGRAM = _build_program()
    nc = _PROGRAM

    mmnp = mybir.dt.np(MMDT)

    in_maps = []
    xT_mm = xT.astype(mmnp)
    pwB_mm = pwB.astype(mmnp)
    for c in range(NCORES):
        cols = slice(c * HPC * D, (c + 1) * HPC * D)
        wqT = effT["q"][:, cols].copy()
        wkT = effT["k"][:, cols].copy()
        for hl in range(HPC):
            sl = slice(hl * D, (hl + 1) * D)
            wqT[:, sl] = wqT[:, sl][:, perm]
            wkT[:, sl] = wkT[:, sl][:, perm]
        in_maps.append({
            "xT": xT_mm,
            "wqT": np.ascontiguousarray(wqT).astype(mmnp),
            "wkT": np.ascontiguousarray(wkT).astype(mmnp),
            "wvT": np.ascontiguousarray(effT["v"][:, cols]).astype(mmnp),
            "pwB": pwB_mm,
            "cosA": cosA,
            "sinA": sinA,
            "lstep": lstep.astype(mmnp),
            "rmask": rmask.astype(mmnp),
            "ident": ident.astype(mmnp),
        })

    res = run_bass_kernel_spmd(nc, in_maps, list(range(NCORES)))

    out = np.empty((BT, C), np.float32)
    for c in range(NCORES):
        oT = res.results[c]["outT"]                    # [2048, 512]
        out[c * 256:(c + 1) * 256, :] = oT[:, 0:256].T             # b = 0
        out[T + c * 256:T + (c + 1) * 256, :] = oT[:, 256:512].T   # b = 1
    return out.reshape(B, T, C)
